# revision 14
# baseline (speedup 1.0000x reference)
"""Trainium2 Bass kernel for nn_DenoisingPotential.

Math: reference iterates x <- x + alpha * grad_phi(x) 10 times where
  grad_phi(x) = -sum_k softmax_k(c_k - 0.5 (x-mu_k)^T P_k (x-mu_k)) P_k (x-mu_k)
with P_k = A_k^T A_k.

When all P_k are equal (P_k == P for all k, which holds for the identity-A
inputs this problem ships), the quadratic term x^T P x is constant across k
and cancels inside the softmax, so with pm_k = P mu_k:
  scores_k = pm_k . x + (c_k - 0.5 mu_k . pm_k)
  w = softmax(scores)
  x_new = (I - alpha P) x + alpha * w @ pm
This turns the update into two tiny matmuls + a 32-way softmax per sample.

Layout per core (batch 8192 = B/8):
  xT packed (128, 4096) f32 in SBUF: rows 0:64 = x^T of samples [0:4096),
  rows 64:128 = x^T of samples [4096:8192).  All matmuls stream 512-column
  chunks; scores/exp/Z/W live in a (128, 2048) packing that carries 4 samples
  per column (2 halves x 2 column-groups) so ACT/DVE use all 128 lanes.
"""

import os
import numpy as np

B = 65536
D = 64
K = 32
N_ITER = 10
N_CORES = 8
BC = B // N_CORES  # 8192 samples per core
HB = BC // 2       # 4096  (xT columns)
QB = BC // 4       # 2048  (score-packing columns)
CH = 512           # matmul / chunk free size

_MODULE_CACHE = {}


def _build_module(scalar_m=True, m_scalar_val=0.9, n_xi=0, gp_mul=True,
                  gp_cast=True):
    """scalar_m: M = m_scalar_val * I, so the x-path update is a fused
    scalar_tensor_tensor drain (exact fp32); otherwise a fp32 matmul with wI.
    n_xi: even when scalar_m, run this many of the 8 x-chunks through the
    wI-matmul + plain-copy drain instead (engine balancing)."""
    import concourse.bacc as bacc
    import concourse.tile as tile
    from concourse import mybir
    from contextlib import ExitStack

    f32 = mybir.dt.float32
    bf16 = mybir.dt.bfloat16
    Exp = mybir.ActivationFunctionType.Exp
    Alu = mybir.AluOpType

    nc = bacc.Bacc()

    x_in = nc.dram_tensor("x", [BC, D], f32, kind="ExternalInput")
    wS_in = nc.dram_tensor("wS", [128, 64], bf16, kind="ExternalInput")
    wZ_in = nc.dram_tensor("wZ", [128, 128], bf16, kind="ExternalInput")
    wVA_in = nc.dram_tensor("wVA", [128, 128], bf16, kind="ExternalInput")
    wVB_in = nc.dram_tensor("wVB", [128, 128], bf16, kind="ExternalInput")
    wI_in = nc.dram_tensor("wI", [128, 128], f32, kind="ExternalInput")
    ident_in = nc.dram_tensor("ident", [128, 128], f32, kind="ExternalInput")
    bias_in = nc.dram_tensor("biasv", [128, 1], f32, kind="ExternalInput")
    out = nc.dram_tensor("out", [BC, D], f32, kind="ExternalOutput")

    with ExitStack() as ctx:
        tc = ctx.enter_context(tile.TileContext(nc))
        consts = ctx.enter_context(tc.tile_pool(name="consts", bufs=1))
        persist = ctx.enter_context(tc.tile_pool(name="persist", bufs=1))
        work = ctx.enter_context(tc.tile_pool(name="work", bufs=2))
        psS = ctx.enter_context(tc.tile_pool(name="psS", bufs=2, space="PSUM"))
        psZ = ctx.enter_context(tc.tile_pool(name="psZ", bufs=1, space="PSUM"))
        psX = ctx.enter_context(tc.tile_pool(name="psX", bufs=2, space="PSUM"))

        # ---- constants to SBUF ----
        wS = consts.tile([128, 64], bf16, tag="wS")
        wZ = consts.tile([128, 128], bf16, tag="wZ")
        wVA = consts.tile([128, 128], bf16, tag="wVA")
        wVB = consts.tile([128, 128], bf16, tag="wVB")
        wI = consts.tile([128, 128], f32, tag="wI")
        ident = consts.tile([128, 128], f32, tag="ident")
        biasv = consts.tile([128, 1], f32, tag="biasv")
        nc.sync.dma_start(wS, wS_in[:, :])
        nc.sync.dma_start(wZ, wZ_in[:, :])
        nc.sync.dma_start(wVA, wVA_in[:, :])
        nc.sync.dma_start(wVB, wVB_in[:, :])
        nc.sync.dma_start(wI, wI_in[:, :])
        nc.sync.dma_start(ident, ident_in[:, :])
        nc.sync.dma_start(biasv, bias_in[:, :])

        # ---- double-buffered resident x^T (fp32 exact + bf16 scores copy) --
        xT0 = persist.tile([128, HB], f32, tag="xT0")
        xT1 = persist.tile([128, HB], f32, tag="xT1")
        xts = [xT0, xT1]
        xB0 = persist.tile([128, HB], bf16, tag="xB0")
        xB1 = persist.tile([128, HB], bf16, tag="xB1")
        xbs = [xB0, xB1]

        def cast_chunk(dst, src, sl, use_gp):
            if use_gp:
                nc.gpsimd.tensor_copy(out=dst[:, sl], in_=src[:, sl])
            else:
                nc.vector.tensor_copy(out=dst[:, sl], in_=src[:, sl])

        # ---- load x and transpose into xT0 ----
        # staging layout: x_nat[:, t, 0:64] = x rows [128t, 128(t+1)) (half 0)
        #                 x_nat[:, t, 64:128] = x rows [4096+128t, ...) (half 1)
        # so one (128,128) transpose yields [x_H0^T ; x_H1^T] stacked on
        # partitions — exactly xT's packing (transpose out must be psum base 0).
        x_nat = persist.tile([128, 32, 128], f32, tag="xnat")
        xr = x_in.rearrange("(h t p) j -> h p t j", h=2, p=128)  # (2,128,32,64)
        for s in range(4):
            tsl = slice(8 * s, 8 * (s + 1))
            nc.sync.dma_start(x_nat[:, tsl, 0:64], xr[0][:, tsl, :])
            nc.sync.dma_start(x_nat[:, tsl, 64:128], xr[1][:, tsl, :])
        for g in range(8):
            pt = psX.tile([128, 512], f32, tag="X")
            for u in range(4):
                t = 4 * g + u          # column block (b = 128*t within half)
                nc.tensor.transpose(pt[:, 128 * u : 128 * (u + 1)],
                                    x_nat[:, t, :], ident)
            gsl = slice(512 * g, 512 * (g + 1))
            if g % 2 == 0:
                nc.scalar.copy(xT0[:, gsl], pt)
            else:
                nc.vector.tensor_copy(xT0[:, gsl], pt)
            cast_chunk(xB0, xT0, gsl, gp_cast)

        # ---- iterations ----
        for it in range(N_ITER):
            xt = xts[it % 2]
            xtn = xts[(it + 1) % 2]
            xb = xbs[it % 2]
            xbn = xbs[(it + 1) % 2]
            E = work.tile([128, QB], bf16, tag="E")
            rz = work.tile([128, QB], f32, tag="rz")
            W = work.tile([128, QB], bf16, tag="W")

            # scores + exp:  E[:, c] carries exp-scores of samples
            # {c, c+4096, c+2048, c+6144} in row groups of 32.
            for i2 in range(2):
                ps = psS.tile([128, 2 * CH], f32, tag="S")
                for h in range(2):
                    c0 = 2 * CH * i2 + CH * h
                    hsl = slice(CH * h, CH * (h + 1))
                    nc.tensor.matmul(ps[0:64, hsl], wS,
                                     xb[:, c0 : c0 + CH],
                                     start=True, stop=True)
                    nc.tensor.matmul(ps[64:128, hsl], wS,
                                     xb[:, QB + c0 : QB + c0 + CH],
                                     start=True, stop=True)
                nc.scalar.activation(E[:, 2 * CH * i2 : 2 * CH * (i2 + 1)], ps,
                                     func=Exp, bias=biasv, scale=1.0)

            # Z (replicated per 32-row group) -> 1/Z -> W = E/Z
            for i2 in range(2):
                sl = slice(2 * CH * i2, 2 * CH * (i2 + 1))
                pz = psZ.tile([128, 2 * CH], f32, tag="Z")
                for h in range(2):
                    c0 = 2 * CH * i2 + CH * h
                    nc.tensor.matmul(pz[:, CH * h : CH * (h + 1)], wZ,
                                     E[:, c0 : c0 + CH], start=True, stop=True)
                nc.vector.reciprocal_approx_fast(rz[:, sl], pz)
                if gp_mul:
                    nc.gpsimd.tensor_mul(W[:, sl], E[:, sl], rz[:, sl])
                else:
                    nc.vector.tensor_mul(W[:, sl], E[:, sl], rz[:, sl])

            # x_new^T = M x^T + alpha * pm^T w
            for cix in range(8):
                csl = slice(CH * cix, CH * (cix + 1))
                px = psX.tile([128, CH], f32, tag="X")
                use_xi = (not scalar_m) or (cix < n_xi)
                if use_xi:
                    nc.tensor.matmul(px, wI, xt[:, csl],
                                     start=True, stop=False)
                wv = wVA if cix < 4 else wVB
                wcol = CH * cix if cix < 4 else CH * cix - QB
                nc.tensor.matmul(px, wv, W[:, wcol : wcol + CH],
                                 start=(not use_xi), stop=True)
                if use_xi:
                    if cix % 2 == 0:
                        nc.scalar.copy(xtn[:, csl], px)
                    else:
                        nc.vector.tensor_copy(xtn[:, csl], px)
                else:
                    # xtn = xt * m + px   (exact fp32 x-path)
                    nc.vector.scalar_tensor_tensor(
                        out=xtn[:, csl], in0=xt[:, csl],
                        scalar=float(m_scalar_val), in1=px,
                        op0=Alu.mult, op1=Alu.add)
                if it != N_ITER - 1:
                    cast_chunk(xbn, xtn, csl, gp_cast)

        # ---- transpose back and store ----
        # inverse: transpose (128,128) column-blocks of xT; each result is
        # [x_H0 block | x_H1 block] side by side -> two DMAs per block group.
        xfin = xts[N_ITER % 2]
        outr = out.rearrange("(h t p) j -> h p t j", h=2, p=128)  # (2,128,32,64)
        for g in range(8):
            po = psX.tile([128, 512], f32, tag="X")
            for u in range(4):
                t = 4 * g + u
                nc.tensor.transpose(po[:, 128 * u : 128 * (u + 1)],
                                    xfin[:, 128 * t : 128 * (t + 1)], ident)
            ost = work.tile([128, 4, 128], f32, tag="ost")
            if g % 2 == 0:
                nc.scalar.copy(ost, po.rearrange("p (u j) -> p u j", u=4))
            else:
                nc.vector.tensor_copy(ost, po.rearrange("p (u j) -> p u j", u=4))
            tsl = slice(4 * g, 4 * (g + 1))
            nc.sync.dma_start(outr[0][:, tsl, :], ost[:, :, 0:64])
            nc.sync.dma_start(outr[1][:, tsl, :], ost[:, :, 64:128])

    nc.finalize()
    return nc


def _host_constants(c, mu, A, alpha):
    """Host-side precompute. Returns None if the equal-P fast path doesn't
    apply, else the dict of constant tensors for the kernel."""
    c = np.asarray(c, np.float32)
    mu = np.asarray(mu, np.float32)
    A = np.asarray(A, np.float32)
    alpha = np.float32(alpha)
    P = np.einsum("kji,kjl->kil", A, A).astype(np.float32)
    if not np.allclose(P, P[0:1], rtol=1e-6, atol=1e-7):
        return None
    P0 = P[0].astype(np.float64)
    mu64 = mu.astype(np.float64)
    pm = (mu64 @ P0.T)                      # (K, D): pm_k = P mu_k (P sym.)
    bias = c.astype(np.float64) - 0.5 * np.einsum("kj,kj->k", mu64, pm)
    M = np.eye(D) - np.float64(alpha) * P0  # (D, D)

    import ml_dtypes
    bf = ml_dtypes.bfloat16

    pmf = pm.astype(np.float32)
    apm = (np.float64(alpha) * pm).astype(np.float32)

    wS = np.zeros((128, 64), np.float32)
    wS[0:64, 0:32] = pmf.T                 # rows j, cols k  (half 0)
    wS[64:128, 32:64] = pmf.T              # half 1

    wZ = np.zeros((128, 128), np.float32)
    for grp in range(4):
        wZ[32 * grp : 32 * (grp + 1), 32 * grp : 32 * (grp + 1)] = 1.0

    wVA = np.zeros((128, 128), np.float32)
    wVA[0:32, 0:64] = apm                  # rows k, cols j
    wVA[32:64, 64:128] = apm
    wVB = np.zeros((128, 128), np.float32)
    wVB[64:96, 0:64] = apm
    wVB[96:128, 64:128] = apm

    wI = np.zeros((128, 128), np.float32)
    wI[0:64, 0:64] = M.T.astype(np.float32)
    wI[64:128, 64:128] = M.T.astype(np.float32)

    ident = np.eye(128, dtype=np.float32)
    biasv = np.tile(bias.astype(np.float32), 4).reshape(128, 1)

    m0 = float(M[0, 0])
    scalar_m = bool(np.allclose(M, m0 * np.eye(D), rtol=0, atol=1e-7))

    tensors = {
        "wS": wS.astype(bf), "wZ": wZ.astype(bf), "wVA": wVA.astype(bf),
        "wVB": wVB.astype(bf), "wI": wI, "ident": ident, "biasv": biasv,
    }
    return tensors, scalar_m, m0


def _numpy_fallback(x, c, mu, A, alpha):
    x = np.asarray(x, np.float32)
    c = np.asarray(c, np.float32)
    mu = np.asarray(mu, np.float32)
    A = np.asarray(A, np.float32)
    P = np.einsum("kji,kjl->kil", A, A).astype(np.float32)
    for _ in range(N_ITER):
        diff = x[:, None, :] - mu[None, :, :]
        Pd = np.einsum("kij,bkj->bki", P, diff)
        quad = np.einsum("bki,bki->bk", diff, Pd)
        s = c[None, :] - 0.5 * quad
        s = s - s.max(axis=1, keepdims=True)
        e = np.exp(s)
        w = e / e.sum(axis=1, keepdims=True)
        grad = -np.einsum("bk,bki->bi", w, Pd)
        x = x + np.float32(alpha) * grad
    return x.astype(np.float32)


def kernel(x, c, mu, A, alpha):
    x = np.ascontiguousarray(np.asarray(x, np.float32))
    host = _host_constants(c, mu, A, alpha)
    if host is None:
        return _numpy_fallback(x, c, mu, A, alpha)
    consts, scalar_m, m0 = host

    from concourse.bass_utils import run_bass_kernel_spmd

    cfg = (
        scalar_m,
        m0,
        int(os.environ.get("KERNEL_N_XI", "0")),
        bool(int(os.environ.get("KERNEL_GP_MUL", "1"))),
        bool(int(os.environ.get("KERNEL_GP_CAST", "1"))),
    )
    if _MODULE_CACHE.get("cfg") != cfg:
        _MODULE_CACHE["nc"] = _build_module(*cfg)
        _MODULE_CACHE["cfg"] = cfg
    nc = _MODULE_CACHE["nc"]

    core_ids = list(range(N_CORES))
    in_maps = []
    for i in core_ids:
        m = {"x": np.ascontiguousarray(x[i * BC : (i + 1) * BC])}
        m.update(consts)
        in_maps.append(m)

    trace = bool(int(os.environ.get("KERNEL_TRACE", "0")))
    res = run_bass_kernel_spmd(nc, in_maps, core_ids, trace=trace)
    kernel.last_results = res
    kernel.last_exec_time_ns = res.exec_time_ns
    outp = np.concatenate([res.results[i]["out"] for i in core_ids], axis=0)
    return outp.astype(np.float32)


kernel.last_exec_time_ns = None
kernel.last_results = None


# revision 22
# speedup vs baseline: 1.1478x; 1.1478x over previous
"""Trainium2 Bass kernel for nn_DenoisingPotential.

Math: reference iterates x <- x + alpha * grad_phi(x) 10 times where
  grad_phi(x) = -sum_k softmax_k(c_k - 0.5 (x-mu_k)^T P_k (x-mu_k)) P_k (x-mu_k)
with P_k = A_k^T A_k.

When all P_k are equal (P_k == P for all k, which holds for the identity-A
inputs this problem ships), the quadratic term x^T P x is constant across k
and cancels inside the softmax, so with pm_k = P mu_k:
  scores_k = pm_k . x + (c_k - 0.5 mu_k . pm_k)
  w = softmax(scores)
  x_new = (I - alpha P) x + alpha * w @ pm
This turns the update into two tiny matmuls + a 32-way softmax per sample.

Layout per core (batch 8192 = B/8):
  xT packed (128, 4096) f32 in SBUF: rows 0:64 = x^T of samples [0:4096),
  rows 64:128 = x^T of samples [4096:8192).  All matmuls stream 512-column
  chunks; scores/exp/Z/W live in a (128, 2048) packing that carries 4 samples
  per column (2 halves x 2 column-groups) so ACT/DVE use all 128 lanes.
"""

import os
import numpy as np

B = 65536
D = 64
K = 32
N_ITER = 10
N_CORES = 8
BC = B // N_CORES  # 8192 samples per core
HB = BC // 2       # 4096  (xT columns)
QB = BC // 4       # 2048  (score-packing columns)
CH = 512           # matmul / chunk free size

_MODULE_CACHE = {}


def _build_module(scalar_m=True, m_scalar_val=0.9, n_xi=0, gp_mul=True,
                  gp_cast=True):
    """scalar_m: M = m_scalar_val * I, so the x-path update is a fused
    scalar_tensor_tensor drain (exact fp32); otherwise a fp32 matmul with wI.
    n_xi: even when scalar_m, run this many of the 8 x-chunks through the
    wI-matmul + plain-copy drain instead (engine balancing)."""
    import concourse.bacc as bacc
    import concourse.tile as tile
    from concourse import mybir
    from contextlib import ExitStack

    f32 = mybir.dt.float32
    bf16 = mybir.dt.bfloat16
    Exp = mybir.ActivationFunctionType.Exp
    Alu = mybir.AluOpType

    nc = bacc.Bacc()

    x_in = nc.dram_tensor("x", [BC, D], f32, kind="ExternalInput")
    wS_in = nc.dram_tensor("wS", [128, 64], bf16, kind="ExternalInput")
    wZ_in = nc.dram_tensor("wZ", [128, 128], bf16, kind="ExternalInput")
    wVA_in = nc.dram_tensor("wVA", [128, 128], bf16, kind="ExternalInput")
    wVB_in = nc.dram_tensor("wVB", [128, 128], bf16, kind="ExternalInput")
    wI_in = nc.dram_tensor("wI", [128, 128], f32, kind="ExternalInput")
    ident_in = nc.dram_tensor("ident", [128, 128], f32, kind="ExternalInput")
    bias_in = nc.dram_tensor("biasv", [128, 1], f32, kind="ExternalInput")
    out = nc.dram_tensor("out", [BC, D], f32, kind="ExternalOutput")

    with ExitStack() as ctx:
        tc = ctx.enter_context(tile.TileContext(nc))
        consts = ctx.enter_context(tc.tile_pool(name="consts", bufs=1))
        persist = ctx.enter_context(tc.tile_pool(name="persist", bufs=1))
        work = ctx.enter_context(tc.tile_pool(name="work", bufs=3))
        psS = ctx.enter_context(tc.tile_pool(name="psS", bufs=2, space="PSUM"))
        psZ = ctx.enter_context(tc.tile_pool(name="psZ", bufs=2, space="PSUM"))
        psX = ctx.enter_context(tc.tile_pool(name="psX", bufs=2, space="PSUM"))

        # ---- constants to SBUF ----
        wS = consts.tile([128, 64], bf16, tag="wS")
        wZ = consts.tile([128, 128], bf16, tag="wZ")
        wVA = consts.tile([128, 128], bf16, tag="wVA")
        wVB = consts.tile([128, 128], bf16, tag="wVB")
        wI = consts.tile([128, 128], f32, tag="wI")
        ident = consts.tile([128, 128], f32, tag="ident")
        biasv = consts.tile([128, 1], f32, tag="biasv")
        nc.sync.dma_start(wS, wS_in[:, :])
        nc.sync.dma_start(wZ, wZ_in[:, :])
        nc.sync.dma_start(wVA, wVA_in[:, :])
        nc.sync.dma_start(wVB, wVB_in[:, :])
        nc.sync.dma_start(wI, wI_in[:, :])
        nc.sync.dma_start(ident, ident_in[:, :])
        nc.sync.dma_start(biasv, bias_in[:, :])

        # ---- double-buffered resident x^T (fp32 exact + bf16 scores copy) --
        xT0 = persist.tile([128, HB], f32, tag="xT0")
        xT1 = persist.tile([128, HB], f32, tag="xT1")
        xts = [xT0, xT1]
        xB0 = persist.tile([128, HB], bf16, tag="xB0")
        xB1 = persist.tile([128, HB], bf16, tag="xB1")
        xbs = [xB0, xB1]

        def cast_chunk(dst, src, sl, use_gp):
            if use_gp:
                nc.gpsimd.tensor_copy(out=dst[:, sl], in_=src[:, sl])
            else:
                nc.vector.tensor_copy(out=dst[:, sl], in_=src[:, sl])

        # ---- load x and transpose into xT0 ----
        # staging layout: x_nat[:, t, 0:64] = x rows [128t, 128(t+1)) (half 0)
        #                 x_nat[:, t, 64:128] = x rows [4096+128t, ...) (half 1)
        # so one (128,128) transpose yields [x_H0^T ; x_H1^T] stacked on
        # partitions — exactly xT's packing (transpose out must be psum base 0).
        x_nat = persist.tile([128, 32, 128], f32, tag="xnat")
        xr = x_in.rearrange("(h t p) j -> h p t j", h=2, p=128)  # (2,128,32,64)
        for s in range(4):
            tsl = slice(8 * s, 8 * (s + 1))
            nc.sync.dma_start(x_nat[:, tsl, 0:64], xr[0][:, tsl, :])
            nc.sync.dma_start(x_nat[:, tsl, 64:128], xr[1][:, tsl, :])
        for g in range(8):
            pt = psX.tile([128, 512], f32, tag="X")
            for u in range(4):
                t = 4 * g + u          # column block (b = 128*t within half)
                nc.tensor.transpose(pt[:, 128 * u : 128 * (u + 1)],
                                    x_nat[:, t, :], ident)
            gsl = slice(512 * g, 512 * (g + 1))
            if g % 2 == 0:
                nc.scalar.copy(xT0[:, gsl], pt)
            else:
                nc.vector.tensor_copy(xT0[:, gsl], pt)
            cast_chunk(xB0, xT0, gsl, gp_cast)

        # ---- iterations ----
        for it in range(N_ITER):
            xt = xts[it % 2]
            xtn = xts[(it + 1) % 2]
            xb = xbs[it % 2]
            xbn = xbs[(it + 1) % 2]
            E = work.tile([128, QB], bf16, tag="E")
            rz = work.tile([128, QB], f32, tag="rz")
            W = work.tile([128, QB], bf16, tag="W")

            # scores + exp:  E[:, c] carries exp-scores of samples
            # {c, c+4096, c+2048, c+6144} in row groups of 32.
            for i2 in range(2):
                ps = psS.tile([128, 2 * CH], f32, tag="S")
                for h in range(2):
                    c0 = 2 * CH * i2 + CH * h
                    hsl = slice(CH * h, CH * (h + 1))
                    nc.tensor.matmul(ps[0:64, hsl], wS,
                                     xb[:, c0 : c0 + CH],
                                     start=True, stop=True)
                    nc.tensor.matmul(ps[64:128, hsl], wS,
                                     xb[:, QB + c0 : QB + c0 + CH],
                                     start=True, stop=True)
                nc.scalar.activation(E[:, 2 * CH * i2 : 2 * CH * (i2 + 1)], ps,
                                     func=Exp, bias=biasv, scale=1.0)

            # Z (replicated per 32-row group) -> 1/Z -> W = E/Z
            for i in range(4):
                sl = slice(CH * i, CH * (i + 1))
                pz = psZ.tile([128, CH], f32, tag="Z")
                nc.tensor.matmul(pz, wZ, E[:, sl], start=True, stop=True)
                nc.vector.reciprocal_approx_fast(rz[:, sl], pz)
                if gp_mul:
                    nc.gpsimd.tensor_mul(W[:, sl], E[:, sl], rz[:, sl])
                else:
                    nc.vector.tensor_mul(W[:, sl], E[:, sl], rz[:, sl])

            # x_new^T = M x^T + alpha * pm^T w
            # order 0,4,1,5,... : next iteration's score matmul for column
            # group i needs BOTH x chunks i and i+4, so pairing them up
            # unblocks iteration t+1 as early as possible.
            for cix in (0, 4, 1, 5, 2, 6, 3, 7):
                csl = slice(CH * cix, CH * (cix + 1))
                px = psX.tile([128, CH], f32, tag="X")
                use_xi = (not scalar_m) or (cix < n_xi)
                if use_xi:
                    nc.tensor.matmul(px, wI, xt[:, csl],
                                     start=True, stop=False)
                wv = wVA if cix < 4 else wVB
                wcol = CH * cix if cix < 4 else CH * cix - QB
                nc.tensor.matmul(px, wv, W[:, wcol : wcol + CH],
                                 start=(not use_xi), stop=True)
                if use_xi:
                    if cix % 2 == 0:
                        nc.scalar.copy(xtn[:, csl], px)
                    else:
                        nc.vector.tensor_copy(xtn[:, csl], px)
                else:
                    # xtn = xt * m + px   (exact fp32 x-path)
                    nc.vector.scalar_tensor_tensor(
                        out=xtn[:, csl], in0=xt[:, csl],
                        scalar=float(m_scalar_val), in1=px,
                        op0=Alu.mult, op1=Alu.add)
                if it != N_ITER - 1:
                    cast_chunk(xbn, xtn, csl, gp_cast)

        # ---- transpose back and store ----
        # inverse: transpose (128,128) column-blocks of xT; each result is
        # [x_H0 block | x_H1 block] side by side -> two DMAs per block group.
        xfin = xts[N_ITER % 2]
        outr = out.rearrange("(h t p) j -> h p t j", h=2, p=128)  # (2,128,32,64)
        for g in range(8):
            po = psX.tile([128, 512], f32, tag="X")
            for u in range(4):
                t = 4 * g + u
                nc.tensor.transpose(po[:, 128 * u : 128 * (u + 1)],
                                    xfin[:, 128 * t : 128 * (t + 1)], ident)
            ost = work.tile([128, 4, 128], f32, tag="ost")
            if g % 2 == 0:
                nc.scalar.copy(ost, po.rearrange("p (u j) -> p u j", u=4))
            else:
                nc.vector.tensor_copy(ost, po.rearrange("p (u j) -> p u j", u=4))
            tsl = slice(4 * g, 4 * (g + 1))
            nc.sync.dma_start(outr[0][:, tsl, :], ost[:, :, 0:64])
            nc.sync.dma_start(outr[1][:, tsl, :], ost[:, :, 64:128])

    nc.finalize()
    return nc


def _host_constants(c, mu, A, alpha):
    """Host-side precompute. Returns None if the equal-P fast path doesn't
    apply, else the dict of constant tensors for the kernel."""
    c = np.asarray(c, np.float32)
    mu = np.asarray(mu, np.float32)
    A = np.asarray(A, np.float32)
    alpha = np.float32(alpha)
    P = np.einsum("kji,kjl->kil", A, A).astype(np.float32)
    if not np.allclose(P, P[0:1], rtol=1e-6, atol=1e-7):
        return None
    P0 = P[0].astype(np.float64)
    mu64 = mu.astype(np.float64)
    pm = (mu64 @ P0.T)                      # (K, D): pm_k = P mu_k (P sym.)
    bias = c.astype(np.float64) - 0.5 * np.einsum("kj,kj->k", mu64, pm)
    M = np.eye(D) - np.float64(alpha) * P0  # (D, D)

    import ml_dtypes
    bf = ml_dtypes.bfloat16

    pmf = pm.astype(np.float32)
    apm = (np.float64(alpha) * pm).astype(np.float32)

    wS = np.zeros((128, 64), np.float32)
    wS[0:64, 0:32] = pmf.T                 # rows j, cols k  (half 0)
    wS[64:128, 32:64] = pmf.T              # half 1

    wZ = np.zeros((128, 128), np.float32)
    for grp in range(4):
        wZ[32 * grp : 32 * (grp + 1), 32 * grp : 32 * (grp + 1)] = 1.0

    wVA = np.zeros((128, 128), np.float32)
    wVA[0:32, 0:64] = apm                  # rows k, cols j
    wVA[32:64, 64:128] = apm
    wVB = np.zeros((128, 128), np.float32)
    wVB[64:96, 0:64] = apm
    wVB[96:128, 64:128] = apm

    wI = np.zeros((128, 128), np.float32)
    wI[0:64, 0:64] = M.T.astype(np.float32)
    wI[64:128, 64:128] = M.T.astype(np.float32)

    ident = np.eye(128, dtype=np.float32)
    biasv = np.tile(bias.astype(np.float32), 4).reshape(128, 1)

    m0 = float(M[0, 0])
    scalar_m = bool(np.allclose(M, m0 * np.eye(D), rtol=0, atol=1e-7))

    tensors = {
        "wS": wS.astype(bf), "wZ": wZ.astype(bf), "wVA": wVA.astype(bf),
        "wVB": wVB.astype(bf), "wI": wI, "ident": ident, "biasv": biasv,
    }
    return tensors, scalar_m, m0


def _numpy_fallback(x, c, mu, A, alpha):
    x = np.asarray(x, np.float32)
    c = np.asarray(c, np.float32)
    mu = np.asarray(mu, np.float32)
    A = np.asarray(A, np.float32)
    P = np.einsum("kji,kjl->kil", A, A).astype(np.float32)
    for _ in range(N_ITER):
        diff = x[:, None, :] - mu[None, :, :]
        Pd = np.einsum("kij,bkj->bki", P, diff)
        quad = np.einsum("bki,bki->bk", diff, Pd)
        s = c[None, :] - 0.5 * quad
        s = s - s.max(axis=1, keepdims=True)
        e = np.exp(s)
        w = e / e.sum(axis=1, keepdims=True)
        grad = -np.einsum("bk,bki->bi", w, Pd)
        x = x + np.float32(alpha) * grad
    return x.astype(np.float32)


def kernel(x, c, mu, A, alpha):
    x = np.ascontiguousarray(np.asarray(x, np.float32))
    host = _host_constants(c, mu, A, alpha)
    if host is None:
        return _numpy_fallback(x, c, mu, A, alpha)
    consts, scalar_m, m0 = host

    from concourse.bass_utils import run_bass_kernel_spmd

    cfg = (
        scalar_m,
        m0,
        int(os.environ.get("KERNEL_N_XI", "2")),
        bool(int(os.environ.get("KERNEL_GP_MUL", "1"))),
        bool(int(os.environ.get("KERNEL_GP_CAST", "1"))),
    )
    if _MODULE_CACHE.get("cfg") != cfg:
        _MODULE_CACHE["nc"] = _build_module(*cfg)
        _MODULE_CACHE["cfg"] = cfg
    nc = _MODULE_CACHE["nc"]

    core_ids = list(range(N_CORES))
    in_maps = []
    for i in core_ids:
        m = {"x": np.ascontiguousarray(x[i * BC : (i + 1) * BC])}
        m.update(consts)
        in_maps.append(m)

    trace = bool(int(os.environ.get("KERNEL_TRACE", "0")))
    res = run_bass_kernel_spmd(nc, in_maps, core_ids, trace=trace)
    kernel.last_results = res
    kernel.last_exec_time_ns = res.exec_time_ns
    outp = np.concatenate([res.results[i]["out"] for i in core_ids], axis=0)
    return outp.astype(np.float32)


kernel.last_exec_time_ns = None
kernel.last_results = None


# revision 26
# speedup vs baseline: 1.2818x; 1.1168x over previous
"""Trainium2 Bass kernel for nn_DenoisingPotential.

Math: reference iterates x <- x + alpha * grad_phi(x) 10 times where
  grad_phi(x) = -sum_k softmax_k(c_k - 0.5 (x-mu_k)^T P_k (x-mu_k)) P_k (x-mu_k)
with P_k = A_k^T A_k.

When all P_k are equal (P_k == P for all k, which holds for the identity-A
inputs this problem ships), the quadratic term x^T P x is constant across k
and cancels inside the softmax, so with pm_k = P mu_k:
  scores_k = pm_k . x + (c_k - 0.5 mu_k . pm_k)
  w = softmax(scores)
  x_new = (I - alpha P) x + alpha * w @ pm
This turns the update into two tiny matmuls + a 32-way softmax per sample.

Layout per core (batch 8192 = B/8):
  xT packed (128, 4096) f32 in SBUF: rows 0:64 = x^T of samples [0:4096),
  rows 64:128 = x^T of samples [4096:8192).  All matmuls stream 512-column
  chunks; scores/exp/Z/W live in a (128, 2048) packing that carries 4 samples
  per column (2 halves x 2 column-groups) so ACT/DVE use all 128 lanes.
"""

import os
import numpy as np

B = 65536
D = 64
K = 32
N_ITER = 10
N_CORES = 8
BC = B // N_CORES  # 8192 samples per core
HB = BC // 2       # 4096  (xT columns)
QB = BC // 4       # 2048  (score-packing columns)
CH = 512           # matmul / chunk free size

_MODULE_CACHE = {}


def _build_module(scalar_m=True, m_scalar_val=0.9, n_xi=0, gp_mul=True,
                  gp_cast=True):
    """scalar_m: M = m_scalar_val * I, so the x-path update is a fused
    scalar_tensor_tensor drain (exact fp32); otherwise a fp32 matmul with wI.
    n_xi: even when scalar_m, run this many of the 8 x-chunks through the
    wI-matmul + plain-copy drain instead (engine balancing)."""
    import concourse.bacc as bacc
    import concourse.tile as tile
    from concourse import mybir
    from contextlib import ExitStack

    f32 = mybir.dt.float32
    bf16 = mybir.dt.bfloat16
    Exp = mybir.ActivationFunctionType.Exp
    Alu = mybir.AluOpType

    nc = bacc.Bacc()

    x_in = nc.dram_tensor("x", [BC, D], f32, kind="ExternalInput")
    wS_in = nc.dram_tensor("wS", [128, 64], bf16, kind="ExternalInput")
    wZ_in = nc.dram_tensor("wZ", [128, 128], bf16, kind="ExternalInput")
    wVA_in = nc.dram_tensor("wVA", [128, 128], bf16, kind="ExternalInput")
    wVB_in = nc.dram_tensor("wVB", [128, 128], bf16, kind="ExternalInput")
    wI_in = nc.dram_tensor("wI", [128, 128], f32, kind="ExternalInput")
    ident_in = nc.dram_tensor("ident", [128, 128], f32, kind="ExternalInput")
    bias_in = nc.dram_tensor("biasv", [128, 1], f32, kind="ExternalInput")
    out = nc.dram_tensor("out", [BC, D], f32, kind="ExternalOutput")

    with ExitStack() as ctx:
        tc = ctx.enter_context(tile.TileContext(nc))
        consts = ctx.enter_context(tc.tile_pool(name="consts", bufs=1))
        persist = ctx.enter_context(tc.tile_pool(name="persist", bufs=1))
        work = ctx.enter_context(tc.tile_pool(name="work", bufs=4))
        psS = ctx.enter_context(tc.tile_pool(name="psS", bufs=3, space="PSUM"))
        psZ = ctx.enter_context(tc.tile_pool(name="psZ", bufs=2, space="PSUM"))
        psX = ctx.enter_context(tc.tile_pool(name="psX", bufs=3, space="PSUM"))

        # ---- constants to SBUF ----
        wS = consts.tile([128, 64], bf16, tag="wS")
        wZ = consts.tile([128, 128], bf16, tag="wZ")
        wVA = consts.tile([128, 128], bf16, tag="wVA")
        wVB = consts.tile([128, 128], bf16, tag="wVB")
        wI = consts.tile([128, 128], f32, tag="wI")
        ident = consts.tile([128, 128], f32, tag="ident")
        biasv = consts.tile([128, 1], f32, tag="biasv")
        nc.sync.dma_start(wS, wS_in[:, :])
        nc.sync.dma_start(wZ, wZ_in[:, :])
        nc.sync.dma_start(wVA, wVA_in[:, :])
        nc.sync.dma_start(wVB, wVB_in[:, :])
        nc.sync.dma_start(wI, wI_in[:, :])
        nc.sync.dma_start(ident, ident_in[:, :])
        nc.sync.dma_start(biasv, bias_in[:, :])

        # ---- double-buffered resident x^T (fp32 exact + bf16 scores copy) --
        xT0 = persist.tile([128, HB], f32, tag="xT0")
        xT1 = persist.tile([128, HB], f32, tag="xT1")
        xts = [xT0, xT1]
        xB0 = persist.tile([128, HB], bf16, tag="xB0")
        xB1 = persist.tile([128, HB], bf16, tag="xB1")
        xbs = [xB0, xB1]

        def cast_chunk(dst, src, sl, use_gp):
            if use_gp:
                nc.gpsimd.tensor_copy(out=dst[:, sl], in_=src[:, sl])
            else:
                nc.vector.tensor_copy(out=dst[:, sl], in_=src[:, sl])

        # ---- load x and transpose into xT0 ----
        # staging layout: x_nat[:, t, 0:64] = x rows [128t, 128(t+1)) (half 0)
        #                 x_nat[:, t, 64:128] = x rows [4096+128t, ...) (half 1)
        # so one (128,128) transpose yields [x_H0^T ; x_H1^T] stacked on
        # partitions — exactly xT's packing (transpose out must be psum base 0).
        x_nat = persist.tile([128, 32, 128], f32, tag="xnat")
        xr = x_in.rearrange("(h t p) j -> h p t j", h=2, p=128)  # (2,128,32,64)
        for s in range(8):
            tsl = slice(4 * s, 4 * (s + 1))
            nc.sync.dma_start(x_nat[:, tsl, 0:64], xr[0][:, tsl, :])
            nc.sync.dma_start(x_nat[:, tsl, 64:128], xr[1][:, tsl, :])
        for g in range(8):
            pt = psX.tile([128, 512], f32, tag="X")
            for u in range(4):
                t = 4 * g + u          # column block (b = 128*t within half)
                nc.tensor.transpose(pt[:, 128 * u : 128 * (u + 1)],
                                    x_nat[:, t, :], ident)
            gsl = slice(512 * g, 512 * (g + 1))
            if g % 2 == 0:
                nc.scalar.copy(xT0[:, gsl], pt)
            else:
                nc.vector.tensor_copy(xT0[:, gsl], pt)
            cast_chunk(xB0, xT0, gsl, gp_cast)

        # ---- iterations ----
        for it in range(N_ITER):
            xt = xts[it % 2]
            xtn = xts[(it + 1) % 2]
            xb = xbs[it % 2]
            xbn = xbs[(it + 1) % 2]
            E = work.tile([128, QB], bf16, tag="E")
            rz = work.tile([128, QB], f32, tag="rz")
            W = work.tile([128, QB], bf16, tag="W")

            # scores + exp:  E[:, c] carries exp-scores of samples
            # {c, c+4096, c+2048, c+6144} in row groups of 32.
            for i in range(4):
                c0 = CH * i
                ps = psS.tile([128, CH], f32, tag="S")
                nc.tensor.matmul(ps[0:64, :], wS,
                                 xb[:, c0 : c0 + CH],
                                 start=True, stop=True)
                nc.tensor.matmul(ps[64:128, :], wS,
                                 xb[:, QB + c0 : QB + c0 + CH],
                                 start=True, stop=True)
                nc.scalar.activation(E[:, c0 : c0 + CH], ps,
                                     func=Exp, bias=biasv, scale=1.0)

            # Z (replicated per 32-row group) -> 1/Z -> W = E/Z
            for i in range(4):
                sl = slice(CH * i, CH * (i + 1))
                pz = psZ.tile([128, CH], f32, tag="Z")
                nc.tensor.matmul(pz, wZ, E[:, sl], start=True, stop=True)
                nc.vector.reciprocal_approx_fast(rz[:, sl], pz)
                if gp_mul:
                    nc.gpsimd.tensor_mul(W[:, sl], E[:, sl], rz[:, sl])
                else:
                    nc.vector.tensor_mul(W[:, sl], E[:, sl], rz[:, sl])

            # x_new^T = M x^T + alpha * pm^T w
            # order 0,4,1,5,... : next iteration's score matmul for column
            # group i needs BOTH x chunks i and i+4, so pairing them up
            # unblocks iteration t+1 as early as possible.
            for cix in (0, 4, 1, 5, 2, 6, 3, 7):
                csl = slice(CH * cix, CH * (cix + 1))
                px = psX.tile([128, CH], f32, tag="X")
                use_xi = (not scalar_m) or (cix < n_xi)
                if use_xi:
                    nc.tensor.matmul(px, wI, xt[:, csl],
                                     start=True, stop=False)
                wv = wVA if cix < 4 else wVB
                wcol = CH * cix if cix < 4 else CH * cix - QB
                nc.tensor.matmul(px, wv, W[:, wcol : wcol + CH],
                                 start=(not use_xi), stop=True)
                if use_xi:
                    if cix % 2 == 0:
                        nc.scalar.copy(xtn[:, csl], px)
                    else:
                        nc.vector.tensor_copy(xtn[:, csl], px)
                else:
                    # xtn = xt * m + px   (exact fp32 x-path)
                    nc.vector.scalar_tensor_tensor(
                        out=xtn[:, csl], in0=xt[:, csl],
                        scalar=float(m_scalar_val), in1=px,
                        op0=Alu.mult, op1=Alu.add)
                if it != N_ITER - 1:
                    cast_chunk(xbn, xtn, csl, gp_cast)

        # ---- transpose back and store ----
        # inverse: transpose (128,128) column-blocks of xT; each result is
        # [x_H0 block | x_H1 block] side by side -> two DMAs per block group.
        xfin = xts[N_ITER % 2]
        outr = out.rearrange("(h t p) j -> h p t j", h=2, p=128)  # (2,128,32,64)
        for g in range(8):
            po = psX.tile([128, 512], f32, tag="X")
            for u in range(4):
                t = 4 * g + u
                nc.tensor.transpose(po[:, 128 * u : 128 * (u + 1)],
                                    xfin[:, 128 * t : 128 * (t + 1)], ident)
            ost = work.tile([128, 4, 128], f32, tag="ost")
            if g % 2 == 0:
                nc.scalar.copy(ost, po.rearrange("p (u j) -> p u j", u=4))
            else:
                nc.vector.tensor_copy(ost, po.rearrange("p (u j) -> p u j", u=4))
            tsl = slice(4 * g, 4 * (g + 1))
            nc.sync.dma_start(outr[0][:, tsl, :], ost[:, :, 0:64])
            nc.sync.dma_start(outr[1][:, tsl, :], ost[:, :, 64:128])

    nc.finalize()
    return nc


def _host_constants(c, mu, A, alpha):
    """Host-side precompute. Returns None if the equal-P fast path doesn't
    apply, else the dict of constant tensors for the kernel."""
    c = np.asarray(c, np.float32)
    mu = np.asarray(mu, np.float32)
    A = np.asarray(A, np.float32)
    alpha = np.float32(alpha)
    P = np.einsum("kji,kjl->kil", A, A).astype(np.float32)
    if not np.allclose(P, P[0:1], rtol=1e-6, atol=1e-7):
        return None
    P0 = P[0].astype(np.float64)
    mu64 = mu.astype(np.float64)
    pm = (mu64 @ P0.T)                      # (K, D): pm_k = P mu_k (P sym.)
    bias = c.astype(np.float64) - 0.5 * np.einsum("kj,kj->k", mu64, pm)
    M = np.eye(D) - np.float64(alpha) * P0  # (D, D)

    import ml_dtypes
    bf = ml_dtypes.bfloat16

    pmf = pm.astype(np.float32)
    apm = (np.float64(alpha) * pm).astype(np.float32)

    wS = np.zeros((128, 64), np.float32)
    wS[0:64, 0:32] = pmf.T                 # rows j, cols k  (half 0)
    wS[64:128, 32:64] = pmf.T              # half 1

    wZ = np.zeros((128, 128), np.float32)
    for grp in range(4):
        wZ[32 * grp : 32 * (grp + 1), 32 * grp : 32 * (grp + 1)] = 1.0

    wVA = np.zeros((128, 128), np.float32)
    wVA[0:32, 0:64] = apm                  # rows k, cols j
    wVA[32:64, 64:128] = apm
    wVB = np.zeros((128, 128), np.float32)
    wVB[64:96, 0:64] = apm
    wVB[96:128, 64:128] = apm

    wI = np.zeros((128, 128), np.float32)
    wI[0:64, 0:64] = M.T.astype(np.float32)
    wI[64:128, 64:128] = M.T.astype(np.float32)

    ident = np.eye(128, dtype=np.float32)
    biasv = np.tile(bias.astype(np.float32), 4).reshape(128, 1)

    m0 = float(M[0, 0])
    scalar_m = bool(np.allclose(M, m0 * np.eye(D), rtol=0, atol=1e-7))

    tensors = {
        "wS": wS.astype(bf), "wZ": wZ.astype(bf), "wVA": wVA.astype(bf),
        "wVB": wVB.astype(bf), "wI": wI, "ident": ident, "biasv": biasv,
    }
    return tensors, scalar_m, m0


def _numpy_fallback(x, c, mu, A, alpha):
    x = np.asarray(x, np.float32)
    c = np.asarray(c, np.float32)
    mu = np.asarray(mu, np.float32)
    A = np.asarray(A, np.float32)
    P = np.einsum("kji,kjl->kil", A, A).astype(np.float32)
    for _ in range(N_ITER):
        diff = x[:, None, :] - mu[None, :, :]
        Pd = np.einsum("kij,bkj->bki", P, diff)
        quad = np.einsum("bki,bki->bk", diff, Pd)
        s = c[None, :] - 0.5 * quad
        s = s - s.max(axis=1, keepdims=True)
        e = np.exp(s)
        w = e / e.sum(axis=1, keepdims=True)
        grad = -np.einsum("bk,bki->bi", w, Pd)
        x = x + np.float32(alpha) * grad
    return x.astype(np.float32)


def kernel(x, c, mu, A, alpha):
    x = np.ascontiguousarray(np.asarray(x, np.float32))
    host = _host_constants(c, mu, A, alpha)
    if host is None:
        return _numpy_fallback(x, c, mu, A, alpha)
    consts, scalar_m, m0 = host

    from concourse.bass_utils import run_bass_kernel_spmd

    cfg = (
        scalar_m,
        m0,
        int(os.environ.get("KERNEL_N_XI", "3")),
        bool(int(os.environ.get("KERNEL_GP_MUL", "1"))),
        bool(int(os.environ.get("KERNEL_GP_CAST", "1"))),
    )
    if _MODULE_CACHE.get("cfg") != cfg:
        _MODULE_CACHE["nc"] = _build_module(*cfg)
        _MODULE_CACHE["cfg"] = cfg
    nc = _MODULE_CACHE["nc"]

    core_ids = list(range(N_CORES))
    in_maps = []
    for i in core_ids:
        m = {"x": np.ascontiguousarray(x[i * BC : (i + 1) * BC])}
        m.update(consts)
        in_maps.append(m)

    trace = bool(int(os.environ.get("KERNEL_TRACE", "0")))
    res = run_bass_kernel_spmd(nc, in_maps, core_ids, trace=trace)
    kernel.last_results = res
    kernel.last_exec_time_ns = res.exec_time_ns
    outp = np.concatenate([res.results[i]["out"] for i in core_ids], axis=0)
    return outp.astype(np.float32)


kernel.last_exec_time_ns = None
kernel.last_results = None


# revision 29
# speedup vs baseline: 1.2955x; 1.0107x over previous
"""Trainium2 Bass kernel for nn_DenoisingPotential.

Math: reference iterates x <- x + alpha * grad_phi(x) 10 times where
  grad_phi(x) = -sum_k softmax_k(c_k - 0.5 (x-mu_k)^T P_k (x-mu_k)) P_k (x-mu_k)
with P_k = A_k^T A_k.

When all P_k are equal (P_k == P for all k, which holds for the identity-A
inputs this problem ships), the quadratic term x^T P x is constant across k
and cancels inside the softmax, so with pm_k = P mu_k:
  scores_k = pm_k . x + (c_k - 0.5 mu_k . pm_k)
  w = softmax(scores)
  x_new = (I - alpha P) x + alpha * w @ pm
This turns the update into two tiny matmuls + a 32-way softmax per sample.

Layout per core (batch 8192 = B/8):
  xT packed (128, 4096) f32 in SBUF: rows 0:64 = x^T of samples [0:4096),
  rows 64:128 = x^T of samples [4096:8192).  All matmuls stream 512-column
  chunks; scores/exp/Z/W live in a (128, 2048) packing that carries 4 samples
  per column (2 halves x 2 column-groups) so ACT/DVE use all 128 lanes.
"""

import os
import numpy as np

B = 65536
D = 64
K = 32
N_ITER = 10
N_CORES = 8
BC = B // N_CORES  # 8192 samples per core
HB = BC // 2       # 4096  (xT columns)
QB = BC // 4       # 2048  (score-packing columns)
CH = 512           # matmul / chunk free size

_MODULE_CACHE = {}


def _build_module(scalar_m=True, m_scalar_val=0.9, n_xi=0, gp_mul=True,
                  gp_cast=True):
    """scalar_m: M = m_scalar_val * I, so the x-path update is a fused
    scalar_tensor_tensor drain (exact fp32); otherwise a fp32 matmul with wI.
    n_xi: even when scalar_m, run this many of the 8 x-chunks through the
    wI-matmul + plain-copy drain instead (engine balancing)."""
    import concourse.bacc as bacc
    import concourse.tile as tile
    from concourse import mybir
    from contextlib import ExitStack

    f32 = mybir.dt.float32
    bf16 = mybir.dt.bfloat16
    Exp = mybir.ActivationFunctionType.Exp
    Alu = mybir.AluOpType

    nc = bacc.Bacc()

    x_in = nc.dram_tensor("x", [BC, D], f32, kind="ExternalInput")
    wS_in = nc.dram_tensor("wS", [128, 64], bf16, kind="ExternalInput")
    wZ_in = nc.dram_tensor("wZ", [128, 128], bf16, kind="ExternalInput")
    wVA_in = nc.dram_tensor("wVA", [128, 128], bf16, kind="ExternalInput")
    wVB_in = nc.dram_tensor("wVB", [128, 128], bf16, kind="ExternalInput")
    wI_in = nc.dram_tensor("wI", [128, 128], f32, kind="ExternalInput")
    ident_in = nc.dram_tensor("ident", [128, 128], f32, kind="ExternalInput")
    bias_in = nc.dram_tensor("biasv", [128, 1], f32, kind="ExternalInput")
    out = nc.dram_tensor("out", [BC, D], f32, kind="ExternalOutput")

    with ExitStack() as ctx:
        tc = ctx.enter_context(tile.TileContext(nc))
        consts = ctx.enter_context(tc.tile_pool(name="consts", bufs=1))
        persist = ctx.enter_context(tc.tile_pool(name="persist", bufs=1))
        work = ctx.enter_context(tc.tile_pool(name="work", bufs=4))
        psS = ctx.enter_context(tc.tile_pool(name="psS", bufs=3, space="PSUM"))
        psZ = ctx.enter_context(tc.tile_pool(name="psZ", bufs=2, space="PSUM"))
        psX = ctx.enter_context(tc.tile_pool(name="psX", bufs=3, space="PSUM"))

        # ---- constants to SBUF ----
        wS = consts.tile([128, 64], bf16, tag="wS")
        wZ = consts.tile([128, 128], bf16, tag="wZ")
        wVA = consts.tile([128, 128], bf16, tag="wVA")
        wVB = consts.tile([128, 128], bf16, tag="wVB")
        wI = consts.tile([128, 128], f32, tag="wI")
        ident = consts.tile([128, 128], f32, tag="ident")
        biasv = consts.tile([128, 1], f32, tag="biasv")
        nc.sync.dma_start(wS, wS_in[:, :])
        nc.sync.dma_start(wZ, wZ_in[:, :])
        nc.sync.dma_start(wVA, wVA_in[:, :])
        nc.sync.dma_start(wVB, wVB_in[:, :])
        nc.sync.dma_start(wI, wI_in[:, :])
        nc.sync.dma_start(ident, ident_in[:, :])
        nc.sync.dma_start(biasv, bias_in[:, :])

        # ---- double-buffered resident x^T (fp32 exact + bf16 scores copy) --
        xT0 = persist.tile([128, HB], f32, tag="xT0")
        xT1 = persist.tile([128, HB], f32, tag="xT1")
        xts = [xT0, xT1]
        xB0 = persist.tile([128, HB], bf16, tag="xB0")
        xB1 = persist.tile([128, HB], bf16, tag="xB1")
        xbs = [xB0, xB1]

        def cast_chunk(dst, src, sl, use_gp):
            if use_gp:
                nc.gpsimd.tensor_copy(out=dst[:, sl], in_=src[:, sl])
            else:
                nc.vector.tensor_copy(out=dst[:, sl], in_=src[:, sl])

        # ---- load x and transpose into xT0 ----
        # staging layout: x_nat[:, t, 0:64] = x rows [128t, 128(t+1)) (half 0)
        #                 x_nat[:, t, 64:128] = x rows [4096+128t, ...) (half 1)
        # so one (128,128) transpose yields [x_H0^T ; x_H1^T] stacked on
        # partitions — exactly xT's packing (transpose out must be psum base 0).
        x_nat = persist.tile([128, 32, 128], f32, tag="xnat")
        xr = x_in.rearrange("(h t p) j -> h p t j", h=2, p=128)  # (2,128,32,64)
        for s in range(8):
            tsl = slice(4 * s, 4 * (s + 1))
            nc.sync.dma_start(x_nat[:, tsl, 0:64], xr[0][:, tsl, :])
            nc.sync.dma_start(x_nat[:, tsl, 64:128], xr[1][:, tsl, :])
        # order 0,4,... so iteration 0's first score matmul (needs cast
        # groups i and i+4) unblocks as early as possible
        for g in (0, 4, 1, 5, 2, 6, 3, 7):
            pt = psX.tile([128, 512], f32, tag="X")
            for u in range(4):
                t = 4 * g + u          # column block (b = 128*t within half)
                nc.tensor.transpose(pt[:, 128 * u : 128 * (u + 1)],
                                    x_nat[:, t, :], ident)
            gsl = slice(512 * g, 512 * (g + 1))
            if g % 2 == 0:
                nc.scalar.copy(xT0[:, gsl], pt)
            else:
                nc.vector.tensor_copy(xT0[:, gsl], pt)
            cast_chunk(xB0, xT0, gsl, gp_cast)

        outr = out.rearrange("(h t p) j -> h p t j", h=2, p=128)  # (2,128,32,64)

        # ---- iterations ----
        for it in range(N_ITER):
            xt = xts[it % 2]
            xtn = xts[(it + 1) % 2]
            xb = xbs[it % 2]
            xbn = xbs[(it + 1) % 2]
            E = work.tile([128, QB], bf16, tag="E")
            rz = work.tile([128, QB], f32, tag="rz")
            W = work.tile([128, QB], bf16, tag="W")

            # scores + exp:  E[:, c] carries exp-scores of samples
            # {c, c+4096, c+2048, c+6144} in row groups of 32.
            for i in range(4):
                c0 = CH * i
                ps = psS.tile([128, CH], f32, tag="S")
                nc.tensor.matmul(ps[0:64, :], wS,
                                 xb[:, c0 : c0 + CH],
                                 start=True, stop=True)
                nc.tensor.matmul(ps[64:128, :], wS,
                                 xb[:, QB + c0 : QB + c0 + CH],
                                 start=True, stop=True)
                nc.scalar.activation(E[:, c0 : c0 + CH], ps,
                                     func=Exp, bias=biasv, scale=1.0)

            # Z (replicated per 32-row group) -> 1/Z -> W = E/Z
            for i in range(4):
                sl = slice(CH * i, CH * (i + 1))
                pz = psZ.tile([128, CH], f32, tag="Z")
                nc.tensor.matmul(pz, wZ, E[:, sl], start=True, stop=True)
                nc.vector.reciprocal_approx_fast(rz[:, sl], pz)
                if gp_mul:
                    nc.gpsimd.tensor_mul(W[:, sl], E[:, sl], rz[:, sl])
                else:
                    nc.vector.tensor_mul(W[:, sl], E[:, sl], rz[:, sl])

            # x_new^T = M x^T + alpha * pm^T w
            # order 0,4,1,5,... : next iteration's score matmul for column
            # group i needs BOTH x chunks i and i+4, so pairing them up
            # unblocks iteration t+1 as early as possible.
            for cix in (0, 4, 1, 5, 2, 6, 3, 7):
                csl = slice(CH * cix, CH * (cix + 1))
                px = psX.tile([128, CH], f32, tag="X")
                use_xi = (not scalar_m) or (cix < n_xi)
                if use_xi:
                    nc.tensor.matmul(px, wI, xt[:, csl],
                                     start=True, stop=False)
                wv = wVA if cix < 4 else wVB
                wcol = CH * cix if cix < 4 else CH * cix - QB
                nc.tensor.matmul(px, wv, W[:, wcol : wcol + CH],
                                 start=(not use_xi), stop=True)
                if use_xi:
                    if cix % 2 == 0:
                        nc.scalar.copy(xtn[:, csl], px)
                    else:
                        nc.vector.tensor_copy(xtn[:, csl], px)
                else:
                    # xtn = xt * m + px   (exact fp32 x-path)
                    nc.vector.scalar_tensor_tensor(
                        out=xtn[:, csl], in0=xt[:, csl],
                        scalar=float(m_scalar_val), in1=px,
                        op0=Alu.mult, op1=Alu.add)
                if it != N_ITER - 1:
                    cast_chunk(xbn, xtn, csl, gp_cast)
                else:
                    # ---- output: transpose back + store, interleaved with
                    # the last iteration's drains (group g needs only x
                    # chunk g, so it can start the moment chunk g lands) --
                    g = cix
                    po = psX.tile([128, 512], f32, tag="X")
                    for u in range(4):
                        t = 4 * g + u
                        nc.tensor.transpose(po[:, 128 * u : 128 * (u + 1)],
                                            xtn[:, 128 * t : 128 * (t + 1)],
                                            ident)
                    ost = work.tile([128, 4, 128], f32, tag="ost")
                    if g % 2 == 0:
                        nc.scalar.copy(ost, po.rearrange("p (u j) -> p u j", u=4))
                    else:
                        nc.vector.tensor_copy(
                            ost, po.rearrange("p (u j) -> p u j", u=4))
                    tsl = slice(4 * g, 4 * (g + 1))
                    nc.sync.dma_start(outr[0][:, tsl, :], ost[:, :, 0:64])
                    nc.sync.dma_start(outr[1][:, tsl, :], ost[:, :, 64:128])

    nc.finalize()
    return nc


def _host_constants(c, mu, A, alpha):
    """Host-side precompute. Returns None if the equal-P fast path doesn't
    apply, else the dict of constant tensors for the kernel."""
    c = np.asarray(c, np.float32)
    mu = np.asarray(mu, np.float32)
    A = np.asarray(A, np.float32)
    alpha = np.float32(alpha)
    P = np.einsum("kji,kjl->kil", A, A).astype(np.float32)
    if not np.allclose(P, P[0:1], rtol=1e-6, atol=1e-7):
        return None
    P0 = P[0].astype(np.float64)
    mu64 = mu.astype(np.float64)
    pm = (mu64 @ P0.T)                      # (K, D): pm_k = P mu_k (P sym.)
    bias = c.astype(np.float64) - 0.5 * np.einsum("kj,kj->k", mu64, pm)
    M = np.eye(D) - np.float64(alpha) * P0  # (D, D)

    import ml_dtypes
    bf = ml_dtypes.bfloat16

    pmf = pm.astype(np.float32)
    apm = (np.float64(alpha) * pm).astype(np.float32)

    wS = np.zeros((128, 64), np.float32)
    wS[0:64, 0:32] = pmf.T                 # rows j, cols k  (half 0)
    wS[64:128, 32:64] = pmf.T              # half 1

    wZ = np.zeros((128, 128), np.float32)
    for grp in range(4):
        wZ[32 * grp : 32 * (grp + 1), 32 * grp : 32 * (grp + 1)] = 1.0

    wVA = np.zeros((128, 128), np.float32)
    wVA[0:32, 0:64] = apm                  # rows k, cols j
    wVA[32:64, 64:128] = apm
    wVB = np.zeros((128, 128), np.float32)
    wVB[64:96, 0:64] = apm
    wVB[96:128, 64:128] = apm

    wI = np.zeros((128, 128), np.float32)
    wI[0:64, 0:64] = M.T.astype(np.float32)
    wI[64:128, 64:128] = M.T.astype(np.float32)

    ident = np.eye(128, dtype=np.float32)
    biasv = np.tile(bias.astype(np.float32), 4).reshape(128, 1)

    m0 = float(M[0, 0])
    scalar_m = bool(np.allclose(M, m0 * np.eye(D), rtol=0, atol=1e-7))

    tensors = {
        "wS": wS.astype(bf), "wZ": wZ.astype(bf), "wVA": wVA.astype(bf),
        "wVB": wVB.astype(bf), "wI": wI, "ident": ident, "biasv": biasv,
    }
    return tensors, scalar_m, m0


def _numpy_fallback(x, c, mu, A, alpha):
    x = np.asarray(x, np.float32)
    c = np.asarray(c, np.float32)
    mu = np.asarray(mu, np.float32)
    A = np.asarray(A, np.float32)
    P = np.einsum("kji,kjl->kil", A, A).astype(np.float32)
    for _ in range(N_ITER):
        diff = x[:, None, :] - mu[None, :, :]
        Pd = np.einsum("kij,bkj->bki", P, diff)
        quad = np.einsum("bki,bki->bk", diff, Pd)
        s = c[None, :] - 0.5 * quad
        s = s - s.max(axis=1, keepdims=True)
        e = np.exp(s)
        w = e / e.sum(axis=1, keepdims=True)
        grad = -np.einsum("bk,bki->bi", w, Pd)
        x = x + np.float32(alpha) * grad
    return x.astype(np.float32)


def kernel(x, c, mu, A, alpha):
    x = np.ascontiguousarray(np.asarray(x, np.float32))
    host = _host_constants(c, mu, A, alpha)
    if host is None:
        return _numpy_fallback(x, c, mu, A, alpha)
    consts, scalar_m, m0 = host

    from concourse.bass_utils import run_bass_kernel_spmd

    cfg = (
        scalar_m,
        m0,
        int(os.environ.get("KERNEL_N_XI", "3")),
        bool(int(os.environ.get("KERNEL_GP_MUL", "1"))),
        bool(int(os.environ.get("KERNEL_GP_CAST", "1"))),
    )
    if _MODULE_CACHE.get("cfg") != cfg:
        _MODULE_CACHE["nc"] = _build_module(*cfg)
        _MODULE_CACHE["cfg"] = cfg
    nc = _MODULE_CACHE["nc"]

    core_ids = list(range(N_CORES))
    in_maps = []
    for i in core_ids:
        m = {"x": np.ascontiguousarray(x[i * BC : (i + 1) * BC])}
        m.update(consts)
        in_maps.append(m)

    trace = bool(int(os.environ.get("KERNEL_TRACE", "0")))
    res = run_bass_kernel_spmd(nc, in_maps, core_ids, trace=trace)
    kernel.last_results = res
    kernel.last_exec_time_ns = res.exec_time_ns
    outp = np.concatenate([res.results[i]["out"] for i in core_ids], axis=0)
    return outp.astype(np.float32)


kernel.last_exec_time_ns = None
kernel.last_results = None
